# revision 3
# baseline (speedup 1.0000x reference)
# Dense-MoE (all experts active) Trainium2 kernel, expert-parallel over 8
# NeuronCores. Each core computes its expert's 2-layer MLP over all tokens:
#   fe_e = gelu(h @ W1[e] + b1[e]) @ (probs[e] * W2[e]) + probs[e] * b2[e]
# then a chunked ReduceScatter(add) across the 8 cores sums the expert
# contributions; the host reassembles the full [B, D] output.
#
# Layout: everything on-chip is computed transposed-activation style.
#   hT   [IN, Btile]  (PE transpose of h tiles)
#   hidT [H, Btile] = W1kT.T @ hT  per 128-row chunk, gelu+b1 via ACT
#   fe   [Btile, D] = hidT_chunk.T @ W2_chunk accumulated over H chunks
# Matmuls run as float32r views of fp32 data (full PE rate at N>=256).
import os
import sys

sys.path.insert(0, "/opt/trn_rl_repo")

import numpy as np

import concourse.mybir as mybir
from concourse import bacc, tile
from concourse import masks

B, E, IN, H, D = 4096, 8, 1024, 2048, 1024
NCORES = 8
P = 128
BT = 512                  # tokens per B-tile
NBT = B // BT             # 8 B-tiles
NSUB = BT // P            # 4 token sub-tiles per B-tile
KC1 = IN // P             # 8 contraction chunks, layer 1
MC1 = H // P              # 16 H chunks
ND = D // 512             # 2 output column slices of 512
RS_ROWS = BT // NCORES    # 64 rows each core receives per B-tile chunk

F32 = mybir.dt.float32

_CACHE = {}


def build(mm_dtype_name="float32r", nbt=NBT, use_collective=True):
    mm_dt = getattr(mybir.dt, mm_dtype_name)
    nc = bacc.Bacc("TRN2", target_bir_lowering=False)

    h = nc.declare_dram_parameter("h", [nbt * BT, IN], F32, isOutput=False)
    w1 = nc.declare_dram_parameter("w1", [IN, H], mm_dt, isOutput=False)
    b1t = nc.declare_dram_parameter("b1t", [P, MC1], F32, isOutput=False)
    w2 = nc.declare_dram_parameter("w2", [H, D], mm_dt, isOutput=False)
    b2b = nc.declare_dram_parameter("b2b", [P, D], F32, isOutput=False)
    out_rows = nbt * RS_ROWS if use_collective else nbt * BT
    out = nc.declare_dram_parameter("out", [out_rows, D], F32, isOutput=True)

    with tile.TileContext(nc) as tc:
        with (
            tc.tile_pool(name="weights", bufs=1) as wpool,
            tc.tile_pool(name="consts", bufs=1) as cpool,
            tc.tile_pool(name="hraw", bufs=2) as hraw_pool,
            tc.tile_pool(name="ht", bufs=2) as ht_pool,
            tc.tile_pool(name="hid", bufs=1) as hid_pool,
            tc.tile_pool(name="fe", bufs=1) as fe_pool,
            tc.tile_pool(name="tp_ps", bufs=2, space="PSUM") as tp_psum,
            tc.tile_pool(name="l1_ps", bufs=2, space="PSUM") as l1_psum,
            tc.tile_pool(name="l2_ps", bufs=4, space="PSUM") as l2_psum,
            tc.tile_pool(name="dram", bufs=2, space="DRAM") as dram_pool,
        ):
            # --- resident weights / constants ---
            w1_sb = wpool.tile([P, KC1 * H], mm_dt, tag="w1")
            for k in range(KC1):
                nc.sync.dma_start(w1_sb[:, k * H:(k + 1) * H], w1[k * P:(k + 1) * P, :])
            w2_sb = wpool.tile([P, MC1 * D], mm_dt, tag="w2")
            for m in range(MC1):
                nc.sync.dma_start(w2_sb[:, m * D:(m + 1) * D], w2[m * P:(m + 1) * P, :])
            b1_sb = cpool.tile([P, MC1], F32, tag="b1")
            nc.sync.dma_start(b1_sb[:], b1t[:])
            b2_sb = cpool.tile([P, D], F32, tag="b2")
            nc.sync.dma_start(b2_sb[:], b2b[:])
            ident = cpool.tile([P, P], F32, tag="ident")
            masks.make_identity(nc, ident[:])

            for t in range(nbt):
                # --- transpose this tile's h rows into hT ---
                # ht chunk k (IN rows k*128..) lives at columns [k*BT, (k+1)*BT)
                ht = ht_pool.tile([P, KC1 * BT], mm_dt, tag="ht")
                for s in range(NSUB):
                    hr = hraw_pool.tile([P, IN], F32, tag="hr")
                    nc.sync.dma_start(hr[:], h[t * BT + s * P: t * BT + (s + 1) * P, :])
                    for k in range(KC1):
                        tp = tp_psum.tile([P, P], F32, tag="tp")
                        nc.tensor.transpose(tp[:], hr[:, k * P:(k + 1) * P], ident[:])
                        nc.vector.tensor_copy(
                            ht[:, k * BT + s * P: k * BT + (s + 1) * P], tp[:]
                        )

                # --- layer 1: hidT chunk m = (W1 block).T @ hT, + b1, gelu ---
                hid = hid_pool.tile([P, MC1 * BT], mm_dt, tag="hid")
                for m in range(MC1):
                    ps = l1_psum.tile([P, BT], F32, tag="l1")
                    for k in range(KC1):
                        nc.tensor.matmul(
                            ps[:],
                            w1_sb[:, k * H + m * P: k * H + (m + 1) * P],
                            ht[:, k * BT:(k + 1) * BT],
                            start=(k == 0),
                            stop=(k == KC1 - 1),
                        )
                    nc.scalar.activation(
                        hid[:, m * BT:(m + 1) * BT],
                        ps[:],
                        mybir.ActivationFunctionType.Gelu,
                        bias=b1_sb[:, m:m + 1],
                        scale=1.0,
                    )

                # --- layer 2: fe[b_sub, dslice] += hidT_chunk.T @ W2_chunk ---
                fe_chunk = dram_pool.tile([BT, D], F32, tag="fe_dram")
                for s in range(NSUB):
                    for d in range(ND):
                        ps2 = l2_psum.tile([P, 512], F32, tag="l2")
                        for m in range(MC1):
                            nc.tensor.matmul(
                                ps2[:],
                                hid[:, m * BT + s * P: m * BT + (s + 1) * P],
                                w2_sb[:, m * D + d * 512: m * D + (d + 1) * 512],
                                start=(m == 0),
                                stop=(m == MC1 - 1),
                            )
                        fe_sb = fe_pool.tile([P, 512], F32, tag="fe_sb")
                        nc.vector.tensor_add(
                            fe_sb[:], ps2[:], b2_sb[:, d * 512:(d + 1) * 512]
                        )
                        nc.sync.dma_start(
                            fe_chunk[s * P:(s + 1) * P, d * 512:(d + 1) * 512],
                            fe_sb[:],
                        )

                if use_collective:
                    # --- chunked ReduceScatter over the 8 cores ---
                    rs_chunk = dram_pool.tile([RS_ROWS, D], F32, tag="rs_dram")
                    nc.gpsimd.collective_compute(
                        "ReduceScatter",
                        mybir.AluOpType.add,
                        replica_groups=[list(range(NCORES))],
                        ins=[fe_chunk[:]],
                        outs=[rs_chunk[:]],
                    )
                    nc.sync.dma_start(
                        out[t * RS_ROWS:(t + 1) * RS_ROWS, :], rs_chunk[:]
                    )
                else:
                    nc.sync.dma_start(out[t * BT:(t + 1) * BT, :], fe_chunk[:])

    nc.finalize()
    return nc


def _get_nc(mm_dtype_name):
    key = mm_dtype_name
    if key not in _CACHE:
        _CACHE[key] = build(mm_dtype_name)
    return _CACHE[key]


def _run(inputs, mm_dtype_name="float32r", trace=False):
    from concourse.bass_utils import run_bass_kernel_spmd

    h = np.ascontiguousarray(np.asarray(inputs["h"], dtype=np.float32))
    gate_logits = np.asarray(inputs["gate_logits"], dtype=np.float64)
    W1 = np.asarray(inputs["W1"], dtype=np.float32)
    b1 = np.asarray(inputs["b1"], dtype=np.float32)
    W2 = np.asarray(inputs["W2"], dtype=np.float32)
    b2 = np.asarray(inputs["b2"], dtype=np.float32)

    # gate: softmax over E (uniform for zero logits); fold into W2/b2 per expert
    z = np.exp(gate_logits - gate_logits.max())
    probs = (z / z.sum()).astype(np.float32)

    in_maps = []
    for e in range(NCORES):
        w1_e = np.ascontiguousarray(W1[e])                       # [IN, H]
        b1t_e = np.ascontiguousarray(b1[e].reshape(MC1, P).T)    # [P, MC1]
        w2_e = np.ascontiguousarray(W2[e] * probs[e])            # [H, D]
        b2b_e = np.ascontiguousarray(
            np.broadcast_to(b2[e] * probs[e], (P, D))
        )
        in_maps.append(
            {"h": h, "w1": w1_e, "b1t": b1t_e, "w2": w2_e, "b2b": b2b_e}
        )

    nc = _get_nc(mm_dtype_name)
    res = run_bass_kernel_spmd(nc, in_maps, list(range(NCORES)), trace=trace)

    # Reassemble: RS chunk t gave core r rows [t*BT + r*64, t*BT + (r+1)*64)
    final = np.empty((B, D), dtype=np.float32)
    for r in range(NCORES):
        o = res.results[r]["out"]
        for t in range(NBT):
            final[t * BT + r * RS_ROWS: t * BT + (r + 1) * RS_ROWS] = (
                o[t * RS_ROWS:(t + 1) * RS_ROWS]
            )
    return final, res


def kernel(**inputs):
    mm_dtype_name = os.environ.get("MOE_MM_DTYPE", "float32r")
    final, _ = _run(inputs, mm_dtype_name=mm_dtype_name, trace=False)
    return final
